# revision 9
# baseline (speedup 1.0000x reference)
"""Trainium2 Bass kernel for nn_ContinousNormalizingFlowRHS.

Computes, for z in R^{B x Z} and scalar time t:
  h0 = tanh(W1*t + B1); h1 = tanh(einsum('knm,km->kn', W2, h0) + B2)
  w_in  = (W3_win  @ h1[0] + b3_win ).reshape(F, Z)
  w_out = (W3_wout @ h1[1] + b3_wout).reshape(F, Z)
  b     =  W3_b    @ h1[2] + b3_b
  gate  = sigmoid(W3_gate @ h1[3] + b3_gate)
  h = tanh(z @ w_in.T + b); dz = (h*gate) @ w_out / F
  trace = ((1-h^2)*gate) @ (sum(w_in*w_out,1)) / F
  out = concat([dz, -trace[:,None]], -1)

Strategy (8 NeuronCores, single SPMD launch):
  Phase 1 (f-sharded): each core streams its 1/8 slice of W3_win/W3_wout
  (16.75 MB bf16 each) in 2 MB chunks and runs the matvec on the PE only
  (FWL stationary loads keep up with the 360 GB/s HBM stream; no DVE path,
  no broadcast DMAs).  The slice is processed in two f-halves; when a half
  finishes, its w_inT block, gate/F-folded w_outT block, and per-f scalars
  (sg, b) are packed into a 66 KB DRAM buffer and AllGathered (~0.5 MB
  total) while the next half still streams.
  Phase 2 (batch-sharded): each core computes its own 1024-row batch shard
  against the full gathered [F, Z] weights, so the output is written
  directly from each core -- no ReduceScatter tail.  Stage-2 work on the
  first gathered half overlaps the second half's weight streaming.
"""

import sys
import types
import numpy as np
import ml_dtypes

BF = ml_dtypes.bfloat16

# problem sizes (hardcoded per contract)
Z = 128
N = 256
F = 2048
B = 8192
N_CORES = 8

FL = F // N_CORES          # f per core (256)
HF = FL // 2               # f per half (128)
RH = HF * Z                # W3 rows per half per matrix (16384)
CW = 16384                 # W3 rows per streamed chunk
BL = B // N_CORES          # batch shard per core (1024)
BC = 512                   # batch columns per stage-2 chunk (one PSUM bank)


def _ensure_ntff_hook():
    """run_bass_kernel_spmd(trace=True) under axon needs antenv.axon_hooks."""
    if 'antenv.axon_hooks' in sys.modules:
        return
    try:
        from trn_agent_boot.trn_boot import _ntff_profile_via_ctypes
        hook = _ntff_profile_via_ctypes('/opt/axon/libaxon_pjrt.so')
    except Exception:
        hook = None
    try:
        import antenv
    except Exception:
        return
    mod = types.ModuleType('antenv.axon_hooks')
    mod.get_axon_ntff_profile_hook = lambda: hook
    mod.set_axon_ntff_profile_hook = lambda h: None
    sys.modules['antenv.axon_hooks'] = mod
    antenv.axon_hooks = mod


def build_module(n_cores=N_CORES, debug=False):
    """Build the Bass module (SPMD program, one per core)."""
    import concourse.tile as tile
    from concourse import bacc, mybir

    F32 = mybir.dt.float32
    BF16 = mybir.dt.bfloat16
    ADD = mybir.AluOpType.add
    BYPASS = mybir.AluOpType.bypass
    TANH = mybir.ActivationFunctionType.Tanh
    SIGM = mybir.ActivationFunctionType.Sigmoid

    ncc = CW // 128          # psum cols per chunk (64)
    n_chunks = RH // CW      # chunks per half per matrix (2)

    nc = bacc.Bacc("TRN2", target_bir_lowering=False, debug=debug,
                   num_devices=n_cores)

    def inp(name, shape, dt):
        return nc.dram_tensor(name, shape, dt, kind="ExternalInput").ap()

    t_ap = inp("t", [1, 1], F32)
    w1_ap = inp("w1c", [128, 8], F32)
    b1_ap = inp("b1c", [128, 8], F32)
    b2_ap = inp("b2c", [128, 8], F32)
    w2t_ap = inp("w2tc", [128, 2048], BF16)
    w3winT_ap = inp("w3winT_sl", [N, 2 * RH], BF16)
    w3woutT_ap = inp("w3woutT_sl", [N, 2 * RH], BF16)
    b3win_ap = inp("b3win_c", [128, FL], F32)
    b3wout_ap = inp("b3wout_c", [128, FL], F32)
    w3bT_ap = inp("w3bT_sl", [N, FL], BF16)
    w3gateT_ap = inp("w3gateT_sl", [N, FL], BF16)
    b3b_ap = inp("b3b_c", [128, 2], F32)
    b3gate_ap = inp("b3gate_c", [128, 2], F32)
    zt_ap = inp("ztb", [128, BL], BF16)
    eye_ap = inp("eyeb", [128, 128], BF16)
    ones_ap = inp("onesb", [128, 1], BF16)
    out_ap = nc.dram_tensor("out", [Z + 1, BL], F32, kind="ExternalOutput").ap()

    with tile.TileContext(nc) as tc:
        with tc.tile_pool(name="persist", bufs=1) as pp, \
             tc.tile_pool(name="stream", bufs=2) as sp, \
             tc.tile_pool(name="work", bufs=3) as wp, \
             tc.tile_pool(name="ps_h", bufs=2, space="PSUM") as ps_h, \
             tc.tile_pool(name="ps_dz", bufs=1, space="PSUM") as ps_dz, \
             tc.tile_pool(name="ps_t2", bufs=1, space="PSUM") as ps_t2, \
             tc.tile_pool(name="ps_prep", bufs=2, space="PSUM") as ps_prep, \
             tc.tile_pool(name="dram", bufs=1, space="DRAM") as dp:

            gbuf = [dp.tile([2 + 2 * 128, 128], BF16, tag=f"gbuf{x}",
                            name=f"gbuf{x}") for x in range(2)]
            abuf = dp.tile([1, 64], BF16, tag="abuf", name="abuf")
            agth = dp.tile([8, 64], BF16, tag="agth", name="agth")
            gath = [dp.tile([n_cores * (2 + 2 * 128), 128], BF16,
                            tag=f"gath{x}", name=f"gath{x}") for x in range(2)]

            # align cores before any real work: absorb kernel-start skew
            # while the weight stream warms up, so the first real AllGather
            # doesn't stall on stragglers.
            az = pp.tile([1, 64], BF16, tag="az")
            nc.gpsimd.memset(az[:], 0.0)
            nc.gpsimd.dma_start(abuf[:], az[:])
            nc.gpsimd.collective_compute(
                "AllGather", BYPASS,
                replica_groups=[list(range(n_cores))],
                ins=[abuf.opt()], outs=[agth.opt()])

            # ---- parameter nets (tiny) ----------------------------------
            t_bc = pp.tile([128, 1], F32, tag="tbc")
            nc.sync.dma_start(t_bc[:], t_ap.broadcast_to([128, 1]))
            w1_sb = pp.tile([128, 8], F32, tag="w1")
            b1_sb = pp.tile([128, 8], F32, tag="b1")
            b2_sb = pp.tile([128, 8], F32, tag="b2")
            w2t_sb = pp.tile([128, 2048], BF16, tag="w2t")
            nc.sync.dma_start(w1_sb[:], w1_ap[:])
            nc.sync.dma_start(b1_sb[:], b1_ap[:])
            nc.sync.dma_start(b2_sb[:], b2_ap[:])
            nc.sync.dma_start(w2t_sb[:], w2t_ap[:])

            h0pre = pp.tile([128, 8], F32, tag="h0pre")
            nc.vector.tensor_scalar_mul(h0pre[:], w1_sb[:], t_bc[:, 0:1])
            nc.vector.tensor_add(h0pre[:], h0pre[:], b1_sb[:])
            h0_sb = pp.tile([128, 8], BF16, tag="h0")
            nc.scalar.activation(h0_sb[:], h0pre[:], TANH)

            ps_h1 = ps_prep.tile([128, 8], F32, tag="prep")
            for k4 in range(4):
                for nb in range(2):
                    c = k4 * 2 + nb
                    for mb in range(2):
                        lhs = w2t_sb[:, k4 * 512 + mb * 256 + nb * 128:
                                     k4 * 512 + mb * 256 + nb * 128 + 128]
                        nc.tensor.matmul(ps_h1[:, c:c + 1], lhs,
                                         h0_sb[:, k4 * 2 + mb:k4 * 2 + mb + 1],
                                         start=(mb == 0), stop=(mb == 1))
            h1pre = pp.tile([128, 8], F32, tag="h1pre")
            h1_sb = pp.tile([128, 8], BF16, tag="h1")
            nc.vector.tensor_add(h1pre[:], ps_h1[:], b2_sb[:])
            nc.scalar.activation(h1_sb[:], h1pre[:], TANH)

            # ---- small persistent loads ---------------------------------
            b3win_sb = pp.tile([128, FL], F32, tag="b3win")
            b3wout_sb = pp.tile([128, FL], F32, tag="b3wout")
            nc.gpsimd.dma_start(b3win_sb[:], b3win_ap[:])
            nc.gpsimd.dma_start(b3wout_sb[:], b3wout_ap[:])
            b3b_sb = pp.tile([128, 2], F32, tag="b3b")
            b3gate_sb = pp.tile([128, 2], F32, tag="b3gate")
            nc.gpsimd.dma_start(b3b_sb[:], b3b_ap[:])
            nc.gpsimd.dma_start(b3gate_sb[:], b3gate_ap[:])
            zt_sb = pp.tile([128, BL], BF16, tag="zt")
            nc.gpsimd.dma_start(zt_sb[:], zt_ap[:])
            eye_sb = pp.tile([128, 128], BF16, tag="eye")
            nc.gpsimd.dma_start(eye_sb[:], eye_ap[:])
            ones_sb = pp.tile([128, 1], BF16, tag="ones")
            nc.gpsimd.dma_start(ones_sb[:], ones_ap[:])

            # ---- heads: b and gate (partitions = f within block) --------
            b_sb = pp.tile([128, 2], F32, tag="bh")
            gate_sb = pp.tile([128, 2], F32, tag="gate")
            gpre = pp.tile([128, 2], F32, tag="gpre")
            for w3hT_ap, bias_sb, dst, net in ((w3bT_ap, b3b_sb, b_sb, 2),
                                               (w3gateT_ap, b3gate_sb, gpre, 3)):
                w3ht = wp.tile([128, 2 * FL], BF16, tag="w3head")
                nc.scalar.dma_start(
                    w3ht[:], w3hT_ap.rearrange("(nb p) fl -> p nb fl", p=128))
                phd = ps_prep.tile([128, 2], F32, tag="prep")
                for a in range(2):
                    for nb in range(2):
                        nc.tensor.matmul(
                            phd[:, a:a + 1],
                            w3ht[:, nb * FL + a * 128:nb * FL + (a + 1) * 128],
                            h1_sb[:, net * 2 + nb:net * 2 + nb + 1],
                            start=(nb == 0), stop=(nb == 1))
                nc.vector.tensor_add(dst[:], phd[:], bias_sb[:])
            nc.scalar.activation(gate_sb[:], gpre[:], SIGM)
            gateF = pp.tile([128, 2], F32, tag="gateF")
            nc.scalar.mul(gateF[:], gate_sb[:], 1.0 / F)

            # ---- phase 1: PE-only sharded matvecs -----------------------
            w_inT_loc = pp.tile([128, FL], BF16, tag="winTl")
            w_outT_loc = pp.tile([128, FL], BF16, tag="woutTl")

            def mv_chunk(x, c, w3T_ap, bias_sb, dst, net):
                off = x * RH + c * CW
                n0 = sp.tile([128, CW], BF16, tag="s0")
                nc.sync.dma_start(n0[:], w3T_ap[0:128, off:off + CW])
                n1 = sp.tile([128, CW], BF16, tag="s1")
                nc.scalar.dma_start(n1[:], w3T_ap[128:256, off:off + CW])
                pw = ps_prep.tile([128, ncc], F32, tag="prep")
                for a in range(ncc):
                    nc.tensor.matmul(pw[:, a:a + 1], n0[:, a * 128:(a + 1) * 128],
                                     h1_sb[:, net * 2:net * 2 + 1],
                                     start=True, stop=False)
                    nc.tensor.matmul(pw[:, a:a + 1], n1[:, a * 128:(a + 1) * 128],
                                     h1_sb[:, net * 2 + 1:net * 2 + 2],
                                     start=False, stop=True)
                col0 = x * HF + c * ncc
                nc.vector.tensor_add(dst[:, col0:col0 + ncc], pw[:],
                                     bias_sb[:, col0:col0 + ncc])

            MATS = ((w3winT_ap, b3win_sb, w_inT_loc, 0),
                    (w3woutT_ap, b3wout_sb, w_outT_loc, 1))

            w_inT_g = [None, None]
            w_outgT_g = [None, None]
            sgb_g = [None, None]
            b32_g = [None, None]

            def pack_half(x):
                c0 = x * HF
                ptr = ps_prep.tile([128, 128], BF16, tag="prep")
                nc.tensor.transpose(ptr[:], w_outT_loc[:, c0:c0 + 128], eye_sb[:])
                wog = wp.tile([128, 128], BF16, tag="wog")
                nc.vector.tensor_scalar_mul(wog[:], ptr[:], gateF[:, x:x + 1])
                pti = ps_prep.tile([128, 128], BF16, tag="prep")
                nc.tensor.transpose(pti[:], w_inT_loc[:, c0:c0 + 128], eye_sb[:])
                wif = wp.tile([128, 128], BF16, tag="wif")
                nc.vector.tensor_copy(wif[:], pti[:])
                prod = wp.tile([128, 128], F32, tag="sprod")
                nc.vector.tensor_mul(prod[:], wif[:], wog[:])
                sgf = wp.tile([128, 1], F32, tag="sgf")
                nc.vector.tensor_reduce(sgf[:], prod[:], mybir.AxisListType.X, ADD)
                sgb_pack = wp.tile([128, 2], BF16, tag="sgbp")
                nc.vector.tensor_copy(sgb_pack[:, 0:1], sgf[:])
                nc.vector.tensor_copy(sgb_pack[:, 1:2], b_sb[:, x:x + 1])
                nc.gpsimd.dma_start(gbuf[x][0:128, :], w_inT_loc[:, c0:c0 + 128])
                nc.gpsimd.dma_start(gbuf[x][128:256, :], wog[:])
                nc.gpsimd.dma_start(gbuf[x][256:258, :].rearrange("r f -> f r"),
                                    sgb_pack[:])
                nc.gpsimd.collective_compute(
                    "AllGather", BYPASS,
                    replica_groups=[list(range(n_cores))],
                    ins=[gbuf[x].opt()], outs=[gath[x].opt()])
                rX = gath[x].rearrange("(k r) f -> r k f", k=n_cores)
                w_inT_g[x] = pp.tile([128, n_cores * 128], BF16, tag=f"wing{x}", name=f"wing{x}")
                nc.gpsimd.dma_start(w_inT_g[x][:], rX[0:128])
                w_outgT_g[x] = pp.tile([128, n_cores * 128], BF16, tag=f"wogg{x}", name=f"wogg{x}")
                nc.gpsimd.dma_start(w_outgT_g[x][:], rX[128:256])
                vX = gath[x].rearrange("(k r) f -> f r k", k=n_cores)
                sgb_g[x] = pp.tile([128, n_cores], BF16, tag=f"sgg{x}", name=f"sgg{x}")
                nc.gpsimd.dma_start(sgb_g[x][:], vX[:, 256, :])
                bb = wp.tile([128, n_cores], BF16, tag=f"bbg{x}")
                nc.gpsimd.dma_start(bb[:], vX[:, 257, :])
                b32_g[x] = pp.tile([128, n_cores], F32, tag=f"b32g{x}", name=f"b32g{x}")
                nc.vector.tensor_copy(b32_g[x][:], bb[:])

            pdz = [ps_dz.tile([128, BC], F32, tag=f"pdz{j}", name=f"pdz{j}")
                   for j in range(2)]
            pt2 = [ps_t2.tile([1, BC], F32, tag=f"pt{j}", name=f"pt{j}")
                   for j in range(2)]

            def stage2(x, j):
                for i in range(n_cores):
                    ph = ps_h.tile([128, BC], F32, tag="ph")
                    nc.tensor.matmul(ph[:], w_inT_g[x][:, i * 128:(i + 1) * 128],
                                     zt_sb[:, j * BC:(j + 1) * BC],
                                     start=True, stop=True)
                    h_bf = wp.tile([128, BC], BF16, tag="hbf")
                    nc.scalar.activation(h_bf[:], ph[:], TANH,
                                         bias=b32_g[x][:, i:i + 1])
                    h2 = wp.tile([128, BC], BF16, tag="h2")
                    nc.vector.tensor_mul(h2[:], h_bf[:], h_bf[:])
                    first = (x == 0 and i == 0)
                    last = (x == 1 and i == n_cores - 1)
                    nc.tensor.matmul(pdz[j][:],
                                     w_outgT_g[x][:, i * 128:(i + 1) * 128],
                                     h_bf[:], start=first, stop=last)
                    nc.tensor.matmul(pt2[j][:], sgb_g[x][:, i:i + 1], h2[:],
                                     start=first, stop=last)

            # half A: stream + matvec, pack, AllGather
            for c in range(n_chunks):
                for m in MATS:
                    mv_chunk(0, c, *m)
            pack_half(0)

            # half B streams while stage-2 on half A fills PE idle time
            mv_chunk(1, 0, *MATS[0])
            stage2(0, 0)
            mv_chunk(1, 0, *MATS[1])
            stage2(0, 1)
            pack_half(1)

            # trace constant: cneg = -sum_f sg  (sg already has gate/F folded)
            sgs = wp.tile([128, 2], F32, tag="sgs")
            for x in range(2):
                nc.vector.tensor_reduce(sgs[:, x:x + 1], sgb_g[x][:],
                                        mybir.AxisListType.X, ADD)
            sgsum = wp.tile([128, 1], F32, tag="sgsum")
            nc.vector.tensor_reduce(sgsum[:], sgs[:], mybir.AxisListType.X, ADD)
            sgsum_bf = wp.tile([128, 1], BF16, tag="sgsbf")
            nc.vector.tensor_copy(sgsum_bf[:], sgsum[:])
            cps = ps_prep.tile([1, 1], F32, tag="prep")
            nc.tensor.matmul(cps[:], sgsum_bf[:], ones_sb[:],
                             start=True, stop=True)
            cneg = pp.tile([1, 1], F32, tag="cneg")
            nc.scalar.mul(cneg[:], cps[:], -1.0)

            # ---- outputs (direct, batch-sharded: no collective) ---------
            for j in range(2):
                stage2(1, j)
                dz_sb = wp.tile([128, BC], F32, tag="dzsb")
                nc.vector.tensor_copy(dz_sb[:], pdz[j][:])
                nc.sync.dma_start(out_ap[0:Z, j * BC:(j + 1) * BC], dz_sb[:])
                tr_sb = wp.tile([1, BC], F32, tag="trsb")
                nc.vector.tensor_scalar_add(tr_sb[:], pt2[j][:], cneg[0:1, 0:1])
                nc.gpsimd.dma_start(out_ap[Z:Z + 1, j * BC:(j + 1) * BC],
                                    tr_sb[:])

    nc.compile()
    return nc


def host_prep(t, z_and_logpz, W1, B1, W2, B2, W3_win, b3_win,
              W3_wout, b3_wout, W3_b, b3_b, W3_gate, b3_gate,
              n_cores=N_CORES):
    """Shard + lay out the numpy inputs into per-core in_maps."""
    rows = FL * Z  # W3 rows per core (32768)

    def col8(x):  # [4, 256] -> [128, 8] with col = k*2 + nb
        return np.ascontiguousarray(
            np.asarray(x, np.float32).reshape(4, 2, 128).transpose(2, 0, 1)
            .reshape(128, 8))

    t_in = np.asarray(t, np.float32).reshape(1, 1)
    w1c = col8(np.asarray(W1, np.float32)[:, :, 0])
    b1c = col8(B1)
    b2c = col8(B2)
    # lhsT tile for h1 net: [m128, (k4, mb, n)] = W2[k4, n, mb*128+m128]
    w2tc = np.ascontiguousarray(
        np.asarray(W2, np.float32).transpose(0, 2, 1)        # [k, m, n]
        .reshape(4, 2, 128, 256).transpose(2, 0, 1, 3).reshape(128, 2048)).astype(BF)
    w3win_bf = np.asarray(W3_win, np.float32).astype(BF)
    w3wout_bf = np.asarray(W3_wout, np.float32).astype(BF)
    w3b_bf = np.asarray(W3_b, np.float32).astype(BF)
    w3gate_bf = np.asarray(W3_gate, np.float32).astype(BF)
    b3win = np.asarray(b3_win, np.float32)
    b3wout = np.asarray(b3_wout, np.float32)
    b3b = np.asarray(b3_b, np.float32)
    b3gate = np.asarray(b3_gate, np.float32)
    z = np.asarray(z_and_logpz, np.float32)[:, :Z]
    ztb = np.ascontiguousarray(z.T).astype(BF)
    eye = np.eye(128, dtype=np.float32).astype(BF)
    ones = np.ones((128, 1), dtype=np.float32).astype(BF)

    in_maps = []
    for k in range(n_cores):
        r0 = k * rows
        f0 = k * FL
        in_maps.append({
            "t": t_in, "w1c": w1c, "b1c": b1c, "b2c": b2c, "w2tc": w2tc,
            "w3winT_sl": np.ascontiguousarray(w3win_bf[r0:r0 + rows].T),
            "w3woutT_sl": np.ascontiguousarray(w3wout_bf[r0:r0 + rows].T),
            "b3win_c": np.ascontiguousarray(
                b3win[r0:r0 + rows].reshape(FL, 128).T),
            "b3wout_c": np.ascontiguousarray(
                b3wout[r0:r0 + rows].reshape(FL, 128).T),
            "w3bT_sl": np.ascontiguousarray(w3b_bf[f0:f0 + FL].T),
            "w3gateT_sl": np.ascontiguousarray(w3gate_bf[f0:f0 + FL].T),
            "b3b_c": np.ascontiguousarray(b3b[f0:f0 + FL].reshape(2, 128).T),
            "b3gate_c": np.ascontiguousarray(
                b3gate[f0:f0 + FL].reshape(2, 128).T),
            "ztb": np.ascontiguousarray(ztb[:, k * BL:(k + 1) * BL]),
            "eyeb": eye, "onesb": ones,
        })
    return in_maps


_NC_CACHE = {}


def kernel(**inputs) -> np.ndarray:
    _ensure_ntff_hook()
    from concourse import bass_utils

    key = "full"
    if key not in _NC_CACHE:
        _NC_CACHE[key] = build_module()
    nc = _NC_CACHE[key]

    in_maps = host_prep(**inputs)
    res = bass_utils.run_bass_kernel_spmd(nc, in_maps, list(range(N_CORES)))
    out = np.empty((B, Z + 1), np.float32)
    for k in range(N_CORES):
        out[k * BL:(k + 1) * BL, :] = res.results[k]["out"].T
    return out


# revision 14
# speedup vs baseline: 1.1711x; 1.1711x over previous
"""Trainium2 Bass kernel for nn_ContinousNormalizingFlowRHS.

Computes, for z in R^{B x Z} and scalar time t:
  h0 = tanh(W1*t + B1); h1 = tanh(einsum('knm,km->kn', W2, h0) + B2)
  w_in  = (W3_win  @ h1[0] + b3_win ).reshape(F, Z)
  w_out = (W3_wout @ h1[1] + b3_wout).reshape(F, Z)
  b     =  W3_b    @ h1[2] + b3_b
  gate  = sigmoid(W3_gate @ h1[3] + b3_gate)
  h = tanh(z @ w_in.T + b); dz = (h*gate) @ w_out / F
  trace = ((1-h^2)*gate) @ (sum(w_in*w_out,1)) / F
  out = concat([dz, -trace[:,None]], -1)

Strategy (8 NeuronCores, single SPMD launch):
  Phase 1 (f-sharded): each core streams its 1/8 slice of W3_win first,
  then W3_wout (16.75 MB bf16 each, 32 KB DMA descriptors) and runs the
  matvecs on the PE only (FWL stationary loads keep up with the HBM
  stream).  The w_inT slice is AllGathered while W3_wout still streams --
  the first collective also absorbs cross-core launch skew off the
  critical path.  The b head is computed redundantly (full W3_b on every
  core) so each core can pre-compute h = tanh(z_loc @ w_inT + b) for ALL
  f-blocks under the tail of the W3_wout stream.  A second small
  AllGather moves the gate/F-folded transposed w_out blocks + per-f trace
  weights; only the dz/trace accumulation matmuls remain after it.
  Phase 2 is batch-sharded: each core writes its own [Z+1, B/8] output
  shard directly -- no ReduceScatter.
"""

import sys
import types
import numpy as np
import ml_dtypes

BF = ml_dtypes.bfloat16

# problem sizes (hardcoded per contract)
Z = 128
N = 256
F = 2048
B = 8192
N_CORES = 8

FL = F // N_CORES          # f per core (256)
RPC = FL * Z               # W3 rows per core per matrix (32768)
CW = 16384                 # W3 rows per streamed chunk (32 KB descriptors)
BL = B // N_CORES          # batch shard per core (1024)
BC = 512                   # batch columns per dz/trace accumulation chunk
NFB = F // 128             # global f-blocks (16)


def _ensure_ntff_hook():
    """run_bass_kernel_spmd(trace=True) under axon needs antenv.axon_hooks."""
    if 'antenv.axon_hooks' in sys.modules:
        return
    try:
        from trn_agent_boot.trn_boot import _ntff_profile_via_ctypes
        hook = _ntff_profile_via_ctypes('/opt/axon/libaxon_pjrt.so')
    except Exception:
        hook = None
    try:
        import antenv
    except Exception:
        return
    mod = types.ModuleType('antenv.axon_hooks')
    mod.get_axon_ntff_profile_hook = lambda: hook
    mod.set_axon_ntff_profile_hook = lambda h: None
    sys.modules['antenv.axon_hooks'] = mod
    antenv.axon_hooks = mod


def build_module(n_cores=N_CORES, debug=False, dump=False):
    """Build the Bass module (SPMD program, one per core)."""
    import concourse.tile as tile
    from concourse import bacc, mybir

    F32 = mybir.dt.float32
    BF16 = mybir.dt.bfloat16
    ADD = mybir.AluOpType.add
    BYPASS = mybir.AluOpType.bypass
    TANH = mybir.ActivationFunctionType.Tanh
    SIGM = mybir.ActivationFunctionType.Sigmoid

    ncc = CW // 128          # psum cols per chunk (128)

    nc = bacc.Bacc("TRN2", target_bir_lowering=False, debug=debug,
                   num_devices=n_cores)

    def inp(name, shape, dt):
        return nc.dram_tensor(name, shape, dt, kind="ExternalInput").ap()

    t_ap = inp("t", [1, 1], F32)
    w1_ap = inp("w1c", [128, 8], F32)
    b1_ap = inp("b1c", [128, 8], F32)
    b2_ap = inp("b2c", [128, 8], F32)
    w2t_ap = inp("w2tc", [128, 2048], BF16)
    w3winT_ap = inp("w3winT_sl", [N, RPC], BF16)
    w3woutT_ap = inp("w3woutT_sl", [N, RPC], BF16)
    b3win_ap = inp("b3win_c", [128, FL], F32)
    b3wout_ap = inp("b3wout_c", [128, FL], F32)
    w3bT_ap = inp("w3bT_full", [N, F], BF16)
    b3b_ap = inp("b3b_full", [128, NFB], F32)
    w3gateT_ap = inp("w3gateT_sl", [N, FL], BF16)
    b3gate_ap = inp("b3gate_c", [128, 2], F32)
    zt_ap = inp("ztb", [128, BL], BF16)
    eye_ap = inp("eyeb", [128, 128], BF16)
    ones_ap = inp("onesb", [128, 1], BF16)
    out_ap = nc.dram_tensor("out", [Z + 1, BL], F32, kind="ExternalOutput").ap()
    if dump:
        d_winT = nc.dram_tensor("d_winT", [128, F], BF16, kind="ExternalOutput").ap()
        d_b = nc.dram_tensor("d_b", [128, NFB], F32, kind="ExternalOutput").ap()
        d_sg = nc.dram_tensor("d_sg", [128, 2 * n_cores], BF16, kind="ExternalOutput").ap()
        d_wog = nc.dram_tensor("d_wog", [128, 2 * n_cores * 128], BF16, kind="ExternalOutput").ap()
        d_h = nc.dram_tensor("d_h", [128, 2 * BL], BF16, kind="ExternalOutput").ap()
        d_winL = nc.dram_tensor("d_winL", [128, FL], BF16, kind="ExternalOutput").ap()
        d_woutL = nc.dram_tensor("d_woutL", [128, FL], BF16, kind="ExternalOutput").ap()

    with tile.TileContext(nc) as tc:
        with tc.tile_pool(name="persist", bufs=1) as pp, \
             tc.tile_pool(name="stream", bufs=2) as sp, \
             tc.tile_pool(name="work", bufs=3) as wp, \
             tc.tile_pool(name="ps_big", bufs=1, space="PSUM") as ps_big, \
             tc.tile_pool(name="ps_dz", bufs=1, space="PSUM") as ps_dz, \
             tc.tile_pool(name="ps_t2", bufs=1, space="PSUM") as ps_t2, \
             tc.tile_pool(name="dram", bufs=1, space="DRAM") as dp:

            # DRAM scratch for the two AllGathers
            gb1 = dp.tile([128, FL], BF16, tag="gb1", name="gb1")
            gt1 = dp.tile([n_cores * 128, FL], BF16, tag="gt1", name="gt1")
            gb2 = dp.tile([2 + FL, 128], BF16, tag="gb2", name="gb2")
            gt2 = dp.tile([n_cores * (2 + FL), 128], BF16, tag="gt2", name="gt2")

            def psA(shape, dt):  # ping/pong 2-bank psum rings
                return ps_big.tile(shape, dt, tag="psA", name="psA")

            def psB(shape, dt):
                return ps_big.tile(shape, dt, tag="psB", name="psB")

            # ---- parameter nets (tiny; loads first on fast sync HWDGE) --
            t_bc = pp.tile([128, 1], F32, tag="tbc")
            nc.sync.dma_start(t_bc[:], t_ap.broadcast_to([128, 1]))
            w1_sb = pp.tile([128, 8], F32, tag="w1")
            b1_sb = pp.tile([128, 8], F32, tag="b1")
            b2_sb = pp.tile([128, 8], F32, tag="b2")
            w2t_sb = pp.tile([128, 2048], BF16, tag="w2t")
            nc.sync.dma_start(w1_sb[:], w1_ap[:])
            nc.sync.dma_start(b1_sb[:], b1_ap[:])
            nc.sync.dma_start(b2_sb[:], b2_ap[:])
            nc.sync.dma_start(w2t_sb[:], w2t_ap[:])
            b3gate_sb = pp.tile([128, 2], F32, tag="b3gate")
            nc.sync.dma_start(b3gate_sb[:], b3gate_ap[:])
            b3b_sb = pp.tile([128, NFB], F32, tag="b3b")
            nc.sync.dma_start(b3b_sb[:], b3b_ap[:])

            # gate head weights early on the scalar HWDGE queue
            w3gt_sb = pp.tile([128, 2 * FL], BF16, tag="w3gt")
            nc.scalar.dma_start(
                w3gt_sb[:], w3gateT_ap.rearrange("(nb p) fl -> p nb fl", p=128))
            w3bt_sb = pp.tile([128, 2 * F], BF16, tag="w3bt")
            nc.scalar.dma_start(
                w3bt_sb[:], w3bT_ap.rearrange("(nb p) f -> p nb f", p=128))

            # small persistent loads on gpsimd (not latency-critical)
            b3win_sb = pp.tile([128, FL], F32, tag="b3win")
            b3wout_sb = pp.tile([128, FL], F32, tag="b3wout")
            nc.gpsimd.dma_start(b3win_sb[:], b3win_ap[:])
            nc.gpsimd.dma_start(b3wout_sb[:], b3wout_ap[:])
            zt_sb = pp.tile([128, BL], BF16, tag="zt")
            nc.gpsimd.dma_start(zt_sb[:], zt_ap[:])
            eye_sb = pp.tile([128, 128], BF16, tag="eye")
            nc.gpsimd.dma_start(eye_sb[:], eye_ap[:])
            ones_sb = pp.tile([128, 1], BF16, tag="ones")
            nc.gpsimd.dma_start(ones_sb[:], ones_ap[:])

            h0pre = pp.tile([128, 8], F32, tag="h0pre")
            nc.vector.tensor_scalar_mul(h0pre[:], w1_sb[:], t_bc[:, 0:1])
            nc.vector.tensor_add(h0pre[:], h0pre[:], b1_sb[:])
            h0_sb = pp.tile([128, 8], BF16, tag="h0")
            nc.scalar.activation(h0_sb[:], h0pre[:], TANH)

            ph1 = psA([128, 8], F32)
            for k4 in range(4):
                for nb in range(2):
                    c = k4 * 2 + nb
                    for mb in range(2):
                        lhs = w2t_sb[:, k4 * 512 + mb * 256 + nb * 128:
                                     k4 * 512 + mb * 256 + nb * 128 + 128]
                        nc.tensor.matmul(ph1[:, c:c + 1], lhs,
                                         h0_sb[:, k4 * 2 + mb:k4 * 2 + mb + 1],
                                         start=(mb == 0), stop=(mb == 1))
            h1pre = pp.tile([128, 8], F32, tag="h1pre")
            h1_sb = pp.tile([128, 8], BF16, tag="h1")
            nc.vector.tensor_add(h1pre[:], ph1[:], b2_sb[:])
            nc.scalar.activation(h1_sb[:], h1pre[:], TANH)

            # ---- heads ---------------------------------------------------
            # local gate (2 blocks)
            gpre = pp.tile([128, 2], F32, tag="gpre")
            phg = psB([128, 2], F32)
            for a in range(2):
                for nb in range(2):
                    nc.tensor.matmul(
                        phg[:, a:a + 1],
                        w3gt_sb[:, nb * FL + a * 128:nb * FL + (a + 1) * 128],
                        h1_sb[:, 6 + nb:7 + nb], start=(nb == 0), stop=(nb == 1))
            nc.vector.tensor_add(gpre[:], phg[:], b3gate_sb[:])
            gate_sb = pp.tile([128, 2], F32, tag="gate")
            nc.scalar.activation(gate_sb[:], gpre[:], SIGM)
            gateF = pp.tile([128, 2], F32, tag="gateF")
            nc.scalar.mul(gateF[:], gate_sb[:], 1.0 / F)
            # full b head (all 16 global blocks, redundant on every core)
            phb = psA([128, NFB], F32)
            for a in range(NFB):
                for nb in range(2):
                    nc.tensor.matmul(
                        phb[:, a:a + 1],
                        w3bt_sb[:, nb * F + a * 128:nb * F + (a + 1) * 128],
                        h1_sb[:, 4 + nb:5 + nb], start=(nb == 0), stop=(nb == 1))
            b_full = pp.tile([128, NFB], F32, tag="bfull")
            nc.vector.tensor_add(b_full[:], phb[:], b3b_sb[:])

            # ---- phase 1: PE-only matvec over streamed W3 slices --------
            w_inT_loc = pp.tile([128, FL], BF16, tag="winTl")
            w_outT_loc = pp.tile([128, FL], BF16, tag="woutTl")

            def mv_chunk(c, w3T_ap, bias_sb, dst, net, ps):
                off = c * CW
                n0 = sp.tile([128, CW], BF16, tag="s0")
                nc.sync.dma_start(n0[:], w3T_ap[0:128, off:off + CW])
                n1 = sp.tile([128, CW], BF16, tag="s1")
                nc.scalar.dma_start(n1[:], w3T_ap[128:256, off:off + CW])
                pw = ps([128, ncc], F32)
                for a in range(ncc):
                    nc.tensor.matmul(pw[:, a:a + 1], n0[:, a * 128:(a + 1) * 128],
                                     h1_sb[:, net * 2:net * 2 + 1],
                                     start=True, stop=False)
                    nc.tensor.matmul(pw[:, a:a + 1], n1[:, a * 128:(a + 1) * 128],
                                     h1_sb[:, net * 2 + 1:net * 2 + 2],
                                     start=False, stop=True)
                col0 = c * ncc
                nc.vector.tensor_add(dst[:, col0:col0 + ncc], pw[:],
                                     bias_sb[:, col0:col0 + ncc])

            # stream + matvec W3_win (chunks 0,1), then pack + AllGather #1
            mv_chunk(0, w3winT_ap, b3win_sb, w_inT_loc, 0, psA)
            mv_chunk(1, w3winT_ap, b3win_sb, w_inT_loc, 0, psB)
            nc.gpsimd.dma_start(gb1[:, :], w_inT_loc[:])
            nc.gpsimd.collective_compute(
                "AllGather", BYPASS, replica_groups=[list(range(n_cores))],
                ins=[gb1.opt()], outs=[gt1.opt()])
            w_inT_full = pp.tile([128, F], BF16, tag="winF")
            nc.gpsimd.dma_start(
                w_inT_full[:], gt1.rearrange("(k z) f -> z k f", k=n_cores))

            # stream + matvec W3_wout; h pre-compute interleaves on the PE
            mv_chunk(0, w3woutT_ap, b3wout_sb, w_outT_loc, 1, psA)

            # h = tanh(z @ w_inT + b) for all f-blocks, both batch halves at
            # once (1024-wide activations out of a 2-bank psum).
            hstore = pp.tile([128, NFB * BL], BF16, tag="hstore")

            def part1(x, ps):
                for k in range(n_cores):
                    blk = k * FL + x * 128
                    ph = ps([128, BL], F32)
                    for j in range(2):
                        nc.tensor.matmul(ph[:, j * BC:(j + 1) * BC],
                                         w_inT_full[:, blk:blk + 128],
                                         zt_sb[:, j * BC:(j + 1) * BC],
                                         start=True, stop=True)
                    idx = x * n_cores + k
                    nc.scalar.activation(hstore[:, idx * BL:(idx + 1) * BL],
                                         ph[:], TANH,
                                         bias=b_full[:, k * 2 + x:k * 2 + x + 1])

            part1(0, psB)
            mv_chunk(1, w3woutT_ap, b3wout_sb, w_outT_loc, 1, psA)
            part1(1, psB)

            # ---- pack + AllGather #2: folded w_out blocks + trace wts ---
            sg_pack = wp.tile([128, 2], BF16, tag="sgp")
            for x in range(2):
                c0 = x * 128
                ptr = psA([128, 128], BF16)
                nc.tensor.transpose(ptr[:], w_outT_loc[:, c0:c0 + 128], eye_sb[:])
                wog = wp.tile([128, 128], BF16, tag="wog")
                nc.vector.tensor_scalar_mul(wog[:], ptr[:], gateF[:, x:x + 1])
                pti = psA([128, 128], BF16)
                nc.tensor.transpose(pti[:], w_inT_loc[:, c0:c0 + 128], eye_sb[:])
                wif = wp.tile([128, 128], BF16, tag="wif")
                nc.vector.tensor_copy(wif[:], pti[:])
                prod = wp.tile([128, 128], F32, tag="sprod")
                nc.vector.tensor_mul(prod[:], wif[:], wog[:])
                sgf = wp.tile([128, 1], F32, tag="sgf")
                nc.vector.tensor_reduce(sgf[:], prod[:], mybir.AxisListType.X, ADD)
                nc.vector.tensor_copy(sg_pack[:, x:x + 1], sgf[:])
                nc.gpsimd.dma_start(gb2[2 + c0:2 + c0 + 128, :], wog[:])
            nc.gpsimd.dma_start(gb2[0:2, :].rearrange("r f -> f r"), sg_pack[:])
            nc.gpsimd.collective_compute(
                "AllGather", BYPASS, replica_groups=[list(range(n_cores))],
                ins=[gb2.opt()], outs=[gt2.opt()])
            v2 = gt2.rearrange("(k r) z -> r k z", k=n_cores)
            w_outgT = [None, None]
            for x in range(2):
                w_outgT[x] = pp.tile([128, n_cores * 128], BF16,
                                     tag=f"wogg{x}", name=f"wogg{x}")
                nc.gpsimd.dma_start(w_outgT[x][:],
                                    v2[2 + x * 128:2 + (x + 1) * 128])
            v2s = gt2.rearrange("(k r) z -> z r k", k=n_cores)
            sg_full = pp.tile([128, 2 * n_cores], BF16, tag="sgfull")
            for x in range(2):  # col = x*8+k
                nc.gpsimd.dma_start(sg_full[:, x * n_cores:(x + 1) * n_cores],
                                    v2s[:, x, :])

            # trace constant cneg = -sum_f sg (sg already gate/F-folded)
            sgs = wp.tile([128, 1], F32, tag="sgs")
            nc.vector.tensor_reduce(sgs[:], sg_full[:], mybir.AxisListType.X, ADD)
            sgs_bf = wp.tile([128, 1], BF16, tag="sgsbf")
            nc.vector.tensor_copy(sgs_bf[:], sgs[:])
            cps = psA([1, 1], F32)
            nc.tensor.matmul(cps[:], sgs_bf[:], ones_sb[:], start=True, stop=True)
            cneg = pp.tile([1, 1], F32, tag="cneg")
            nc.scalar.mul(cneg[:], cps[:], -1.0)

            # ---- phase 2 tail: dz / trace accumulation ------------------
            pdz = [ps_dz.tile([128, BC], F32, tag=f"pdz{j}", name=f"pdz{j}")
                   for j in range(2)]
            pt2 = [ps_t2.tile([1, BC], F32, tag=f"pt{j}", name=f"pt{j}")
                   for j in range(2)]
            for x in range(2):
                for k in range(n_cores):
                    idx = x * n_cores + k
                    first = (x == 0 and k == 0)
                    last = (x == 1 and k == n_cores - 1)
                    for j in range(2):
                        hsl = hstore[:, idx * BL + j * BC:idx * BL + (j + 1) * BC]
                        h2 = wp.tile([128, BC], BF16, tag="h2")
                        nc.vector.tensor_mul(h2[:], hsl, hsl)
                        nc.tensor.matmul(pdz[j][:],
                                         w_outgT[x][:, k * 128:(k + 1) * 128],
                                         hsl, start=first, stop=last)
                        nc.tensor.matmul(pt2[j][:],
                                         sg_full[:, x * n_cores + k:
                                                 x * n_cores + k + 1],
                                         h2[:], start=first, stop=last)

            if dump:
                nc.sync.dma_start(d_winT[:], w_inT_full[:])
                nc.sync.dma_start(d_b[:], b_full[:])
                nc.sync.dma_start(d_sg[:], sg_full[:])
                nc.sync.dma_start(d_wog[:, 0:1024], w_outgT[0][:])
                nc.sync.dma_start(d_wog[:, 1024:2048], w_outgT[1][:])
                nc.sync.dma_start(d_h[:, 0:BL], hstore[:, 0:BL])
                nc.sync.dma_start(d_h[:, BL:2 * BL],
                                  hstore[:, n_cores * BL:(n_cores + 1) * BL])
                nc.sync.dma_start(d_winL[:], w_inT_loc[:])
                nc.sync.dma_start(d_woutL[:], w_outT_loc[:])
            for j in range(2):
                dz_sb = wp.tile([128, BC], F32, tag="dzsb")
                nc.vector.tensor_copy(dz_sb[:], pdz[j][:])
                nc.sync.dma_start(out_ap[0:Z, j * BC:(j + 1) * BC], dz_sb[:])
                tr_sb = wp.tile([1, BC], F32, tag="trsb")
                nc.vector.tensor_scalar_add(tr_sb[:], pt2[j][:], cneg[0:1, 0:1])
                nc.gpsimd.dma_start(out_ap[Z:Z + 1, j * BC:(j + 1) * BC],
                                    tr_sb[:])

    nc.compile()
    return nc


def host_prep(t, z_and_logpz, W1, B1, W2, B2, W3_win, b3_win,
              W3_wout, b3_wout, W3_b, b3_b, W3_gate, b3_gate,
              n_cores=N_CORES):
    """Shard + lay out the numpy inputs into per-core in_maps."""

    def col8(x):  # [4, 256] -> [128, 8] with col = k*2 + nb
        return np.ascontiguousarray(
            np.asarray(x, np.float32).reshape(4, 2, 128).transpose(2, 0, 1)
            .reshape(128, 8))

    t_in = np.asarray(t, np.float32).reshape(1, 1)
    w1c = col8(np.asarray(W1, np.float32)[:, :, 0])
    b1c = col8(B1)
    b2c = col8(B2)
    w2tc = np.ascontiguousarray(
        np.asarray(W2, np.float32).transpose(0, 2, 1)
        .reshape(4, 2, 128, 256).transpose(2, 0, 1, 3).reshape(128, 2048)).astype(BF)
    w3win_bf = np.asarray(W3_win, np.float32).astype(BF)
    w3wout_bf = np.asarray(W3_wout, np.float32).astype(BF)
    w3b_full = np.ascontiguousarray(np.asarray(W3_b, np.float32).astype(BF).T)
    b3b_full = np.ascontiguousarray(
        np.asarray(b3_b, np.float32).reshape(NFB, 128).T)
    w3gate_bf = np.asarray(W3_gate, np.float32).astype(BF)
    b3win = np.asarray(b3_win, np.float32)
    b3wout = np.asarray(b3_wout, np.float32)
    b3gate = np.asarray(b3_gate, np.float32)
    z = np.asarray(z_and_logpz, np.float32)[:, :Z]
    ztb = np.ascontiguousarray(z.T).astype(BF)
    eye = np.eye(128, dtype=np.float32).astype(BF)
    ones = np.ones((128, 1), dtype=np.float32).astype(BF)

    in_maps = []
    for k in range(n_cores):
        r0 = k * RPC
        f0 = k * FL
        in_maps.append({
            "t": t_in, "w1c": w1c, "b1c": b1c, "b2c": b2c, "w2tc": w2tc,
            "w3winT_sl": np.ascontiguousarray(w3win_bf[r0:r0 + RPC].T),
            "w3woutT_sl": np.ascontiguousarray(w3wout_bf[r0:r0 + RPC].T),
            "b3win_c": np.ascontiguousarray(
                b3win[r0:r0 + RPC].reshape(FL, 128).T),
            "b3wout_c": np.ascontiguousarray(
                b3wout[r0:r0 + RPC].reshape(FL, 128).T),
            "w3bT_full": w3b_full, "b3b_full": b3b_full,
            "w3gateT_sl": np.ascontiguousarray(w3gate_bf[f0:f0 + FL].T),
            "b3gate_c": np.ascontiguousarray(
                b3gate[f0:f0 + FL].reshape(2, 128).T),
            "ztb": np.ascontiguousarray(ztb[:, k * BL:(k + 1) * BL]),
            "eyeb": eye, "onesb": ones,
        })
    return in_maps


_NC_CACHE = {}


def kernel(**inputs) -> np.ndarray:
    _ensure_ntff_hook()
    from concourse import bass_utils

    key = "full"
    if key not in _NC_CACHE:
        _NC_CACHE[key] = build_module()
    nc = _NC_CACHE[key]

    in_maps = host_prep(**inputs)
    res = bass_utils.run_bass_kernel_spmd(nc, in_maps, list(range(N_CORES)))
    out = np.empty((B, Z + 1), np.float32)
    for k in range(N_CORES):
        out[k * BL:(k + 1) * BL, :] = res.results[k]["out"].T
    return out


# revision 15
# speedup vs baseline: 1.2055x; 1.0294x over previous
"""Trainium2 Bass kernel for nn_ContinousNormalizingFlowRHS.

Computes, for z in R^{B x Z} and scalar time t:
  h0 = tanh(W1*t + B1); h1 = tanh(einsum('knm,km->kn', W2, h0) + B2)
  w_in  = (W3_win  @ h1[0] + b3_win ).reshape(F, Z)
  w_out = (W3_wout @ h1[1] + b3_wout).reshape(F, Z)
  b     =  W3_b    @ h1[2] + b3_b
  gate  = sigmoid(W3_gate @ h1[3] + b3_gate)
  h = tanh(z @ w_in.T + b); dz = (h*gate) @ w_out / F
  trace = ((1-h^2)*gate) @ (sum(w_in*w_out,1)) / F
  out = concat([dz, -trace[:,None]], -1)

Strategy (8 NeuronCores, single SPMD launch):
  Phase 1 (f-sharded): each core streams its 1/8 slice of W3_win first,
  then W3_wout (16.75 MB bf16 each, 32 KB DMA descriptors) and runs the
  matvecs on the PE only (FWL stationary loads keep up with the HBM
  stream).  The w_inT slice is AllGathered while W3_wout still streams --
  the first collective also absorbs cross-core launch skew off the
  critical path.  The b head is computed redundantly (full W3_b on every
  core) so each core can pre-compute h = tanh(z_loc @ w_inT + b) for ALL
  f-blocks under the tail of the W3_wout stream.  A second small
  AllGather moves the gate/F-folded transposed w_out blocks + per-f trace
  weights; only the dz/trace accumulation matmuls remain after it.
  Phase 2 is batch-sharded: each core writes its own [Z+1, B/8] output
  shard directly -- no ReduceScatter.
"""

import sys
import types
import numpy as np
import ml_dtypes

BF = ml_dtypes.bfloat16

# problem sizes (hardcoded per contract)
Z = 128
N = 256
F = 2048
B = 8192
N_CORES = 8

FL = F // N_CORES          # f per core (256)
RPC = FL * Z               # W3 rows per core per matrix (32768)
CW = 16384                 # W3 rows per streamed chunk (32 KB descriptors)
BL = B // N_CORES          # batch shard per core (1024)
BC = 512                   # batch columns per dz/trace accumulation chunk
NFB = F // 128             # global f-blocks (16)


def _ensure_ntff_hook():
    """run_bass_kernel_spmd(trace=True) under axon needs antenv.axon_hooks."""
    if 'antenv.axon_hooks' in sys.modules:
        return
    try:
        from trn_agent_boot.trn_boot import _ntff_profile_via_ctypes
        hook = _ntff_profile_via_ctypes('/opt/axon/libaxon_pjrt.so')
    except Exception:
        hook = None
    try:
        import antenv
    except Exception:
        return
    mod = types.ModuleType('antenv.axon_hooks')
    mod.get_axon_ntff_profile_hook = lambda: hook
    mod.set_axon_ntff_profile_hook = lambda h: None
    sys.modules['antenv.axon_hooks'] = mod
    antenv.axon_hooks = mod


def build_module(n_cores=N_CORES, debug=False, dump=False):
    """Build the Bass module (SPMD program, one per core)."""
    import concourse.tile as tile
    from concourse import bacc, mybir

    F32 = mybir.dt.float32
    BF16 = mybir.dt.bfloat16
    ADD = mybir.AluOpType.add
    BYPASS = mybir.AluOpType.bypass
    TANH = mybir.ActivationFunctionType.Tanh
    SIGM = mybir.ActivationFunctionType.Sigmoid

    ncc = CW // 128          # psum cols per chunk (128)

    nc = bacc.Bacc("TRN2", target_bir_lowering=False, debug=debug,
                   num_devices=n_cores)

    def inp(name, shape, dt):
        return nc.dram_tensor(name, shape, dt, kind="ExternalInput").ap()

    t_ap = inp("t", [1, 1], F32)
    w1_ap = inp("w1c", [128, 8], F32)
    b1_ap = inp("b1c", [128, 8], F32)
    b2_ap = inp("b2c", [128, 8], F32)
    w2t_ap = inp("w2tc", [128, 2048], BF16)
    w3winT_ap = inp("w3winT_sl", [N, RPC], BF16)
    w3woutT_ap = inp("w3woutT_sl", [N, RPC], BF16)
    b3win_ap = inp("b3win_c", [128, FL], F32)
    b3wout_ap = inp("b3wout_c", [128, FL], F32)
    w3bT_ap = inp("w3bT_full", [N, F], BF16)
    b3b_ap = inp("b3b_full", [128, NFB], F32)
    w3gateT_ap = inp("w3gateT_sl", [N, FL], BF16)
    b3gate_ap = inp("b3gate_c", [128, 2], F32)
    zt_ap = inp("ztb", [128, BL], BF16)
    eye_ap = inp("eyeb", [128, 128], BF16)
    ones_ap = inp("onesb", [128, 1], BF16)
    out_ap = nc.dram_tensor("out", [Z + 1, BL], F32, kind="ExternalOutput").ap()
    if dump:
        d_winT = nc.dram_tensor("d_winT", [128, F], BF16, kind="ExternalOutput").ap()
        d_b = nc.dram_tensor("d_b", [128, NFB], F32, kind="ExternalOutput").ap()
        d_sg = nc.dram_tensor("d_sg", [128, 2 * n_cores], BF16, kind="ExternalOutput").ap()
        d_wog = nc.dram_tensor("d_wog", [128, 2 * n_cores * 128], BF16, kind="ExternalOutput").ap()
        d_h = nc.dram_tensor("d_h", [128, 2 * BL], BF16, kind="ExternalOutput").ap()
        d_winL = nc.dram_tensor("d_winL", [128, FL], BF16, kind="ExternalOutput").ap()
        d_woutL = nc.dram_tensor("d_woutL", [128, FL], BF16, kind="ExternalOutput").ap()

    with tile.TileContext(nc) as tc:
        with tc.tile_pool(name="persist", bufs=1) as pp, \
             tc.tile_pool(name="stream", bufs=2) as sp, \
             tc.tile_pool(name="work", bufs=3) as wp, \
             tc.tile_pool(name="ps_big", bufs=1, space="PSUM") as ps_big, \
             tc.tile_pool(name="ps_dz", bufs=1, space="PSUM") as ps_dz, \
             tc.tile_pool(name="ps_t2", bufs=1, space="PSUM") as ps_t2, \
             tc.tile_pool(name="dram", bufs=1, space="DRAM") as dp:

            # DRAM scratch for the two AllGathers
            gb1 = dp.tile([128, FL], BF16, tag="gb1", name="gb1")
            gt1 = dp.tile([n_cores * 128, FL], BF16, tag="gt1", name="gt1")
            gb2 = dp.tile([2 + FL, 128], BF16, tag="gb2", name="gb2")
            gt2 = dp.tile([n_cores * (2 + FL), 128], BF16, tag="gt2", name="gt2")

            def psA(shape, dt):  # ping/pong 2-bank psum rings
                return ps_big.tile(shape, dt, tag="psA", name="psA")

            def psB(shape, dt):
                return ps_big.tile(shape, dt, tag="psB", name="psB")

            # ---- parameter nets (tiny; loads first on fast sync HWDGE) --
            t_bc = pp.tile([128, 1], F32, tag="tbc")
            nc.sync.dma_start(t_bc[:], t_ap.broadcast_to([128, 1]))
            w1_sb = pp.tile([128, 8], F32, tag="w1")
            b1_sb = pp.tile([128, 8], F32, tag="b1")
            b2_sb = pp.tile([128, 8], F32, tag="b2")
            w2t_sb = pp.tile([128, 2048], BF16, tag="w2t")
            nc.sync.dma_start(w1_sb[:], w1_ap[:])
            nc.sync.dma_start(b1_sb[:], b1_ap[:])
            nc.sync.dma_start(b2_sb[:], b2_ap[:])
            nc.sync.dma_start(w2t_sb[:], w2t_ap[:])
            b3gate_sb = pp.tile([128, 2], F32, tag="b3gate")
            nc.sync.dma_start(b3gate_sb[:], b3gate_ap[:])
            b3b_sb = pp.tile([128, NFB], F32, tag="b3b")
            nc.sync.dma_start(b3b_sb[:], b3b_ap[:])

            # gate head weights early on the scalar HWDGE queue
            w3gt_sb = pp.tile([128, 2 * FL], BF16, tag="w3gt")
            nc.scalar.dma_start(
                w3gt_sb[:], w3gateT_ap.rearrange("(nb p) fl -> p nb fl", p=128))
            w3bt_sb = pp.tile([128, 2 * F], BF16, tag="w3bt")
            nc.scalar.dma_start(
                w3bt_sb[:], w3bT_ap.rearrange("(nb p) f -> p nb f", p=128))

            # small persistent loads on gpsimd (not latency-critical)
            b3win_sb = pp.tile([128, FL], F32, tag="b3win")
            b3wout_sb = pp.tile([128, FL], F32, tag="b3wout")
            nc.gpsimd.dma_start(b3win_sb[:], b3win_ap[:])
            nc.gpsimd.dma_start(b3wout_sb[:], b3wout_ap[:])
            zt_sb = pp.tile([128, BL], BF16, tag="zt")
            nc.gpsimd.dma_start(zt_sb[:], zt_ap[:])
            eye_sb = pp.tile([128, 128], BF16, tag="eye")
            nc.gpsimd.dma_start(eye_sb[:], eye_ap[:])
            ones_sb = pp.tile([128, 1], BF16, tag="ones")
            nc.gpsimd.dma_start(ones_sb[:], ones_ap[:])

            h0pre = pp.tile([128, 8], F32, tag="h0pre")
            nc.vector.tensor_scalar_mul(h0pre[:], w1_sb[:], t_bc[:, 0:1])
            nc.vector.tensor_add(h0pre[:], h0pre[:], b1_sb[:])
            h0_sb = pp.tile([128, 8], BF16, tag="h0")
            nc.scalar.activation(h0_sb[:], h0pre[:], TANH)

            ph1 = psA([128, 8], F32)
            for k4 in range(4):
                for nb in range(2):
                    c = k4 * 2 + nb
                    for mb in range(2):
                        lhs = w2t_sb[:, k4 * 512 + mb * 256 + nb * 128:
                                     k4 * 512 + mb * 256 + nb * 128 + 128]
                        nc.tensor.matmul(ph1[:, c:c + 1], lhs,
                                         h0_sb[:, k4 * 2 + mb:k4 * 2 + mb + 1],
                                         start=(mb == 0), stop=(mb == 1))
            h1pre = pp.tile([128, 8], F32, tag="h1pre")
            h1_sb = pp.tile([128, 8], BF16, tag="h1")
            nc.vector.tensor_add(h1pre[:], ph1[:], b2_sb[:])
            nc.scalar.activation(h1_sb[:], h1pre[:], TANH)

            # ---- heads ---------------------------------------------------
            # local gate (2 blocks)
            gpre = pp.tile([128, 2], F32, tag="gpre")
            phg = psB([128, 2], F32)
            for a in range(2):
                for nb in range(2):
                    nc.tensor.matmul(
                        phg[:, a:a + 1],
                        w3gt_sb[:, nb * FL + a * 128:nb * FL + (a + 1) * 128],
                        h1_sb[:, 6 + nb:7 + nb], start=(nb == 0), stop=(nb == 1))
            nc.vector.tensor_add(gpre[:], phg[:], b3gate_sb[:])
            gate_sb = pp.tile([128, 2], F32, tag="gate")
            nc.scalar.activation(gate_sb[:], gpre[:], SIGM)
            gateF = pp.tile([128, 2], F32, tag="gateF")
            nc.scalar.mul(gateF[:], gate_sb[:], 1.0 / F)
            # full b head (all 16 global blocks, redundant on every core)
            phb = psA([128, NFB], F32)
            for a in range(NFB):
                for nb in range(2):
                    nc.tensor.matmul(
                        phb[:, a:a + 1],
                        w3bt_sb[:, nb * F + a * 128:nb * F + (a + 1) * 128],
                        h1_sb[:, 4 + nb:5 + nb], start=(nb == 0), stop=(nb == 1))
            b_full = pp.tile([128, NFB], F32, tag="bfull")
            nc.vector.tensor_add(b_full[:], phb[:], b3b_sb[:])

            # ---- phase 1: PE-only matvec over streamed W3 slices --------
            w_inT_loc = pp.tile([128, FL], BF16, tag="winTl")
            w_outT_loc = pp.tile([128, FL], BF16, tag="woutTl")

            def mv_chunk(c, w3T_ap, bias_sb, dst, net, ps):
                off = c * CW
                n0 = sp.tile([128, CW], BF16, tag="s0")
                nc.sync.dma_start(n0[:], w3T_ap[0:128, off:off + CW])
                n1 = sp.tile([128, CW], BF16, tag="s1")
                nc.scalar.dma_start(n1[:], w3T_ap[128:256, off:off + CW])
                pw = ps([128, ncc], F32)
                for a in range(ncc):
                    nc.tensor.matmul(pw[:, a:a + 1], n0[:, a * 128:(a + 1) * 128],
                                     h1_sb[:, net * 2:net * 2 + 1],
                                     start=True, stop=False)
                    nc.tensor.matmul(pw[:, a:a + 1], n1[:, a * 128:(a + 1) * 128],
                                     h1_sb[:, net * 2 + 1:net * 2 + 2],
                                     start=False, stop=True)
                col0 = c * ncc
                nc.vector.tensor_add(dst[:, col0:col0 + ncc], pw[:],
                                     bias_sb[:, col0:col0 + ncc])

            # stream + matvec W3_win (chunks 0,1), then pack + AllGather #1
            mv_chunk(0, w3winT_ap, b3win_sb, w_inT_loc, 0, psA)
            mv_chunk(1, w3winT_ap, b3win_sb, w_inT_loc, 0, psB)
            nc.gpsimd.dma_start(gb1[:, :], w_inT_loc[:])
            nc.gpsimd.collective_compute(
                "AllGather", BYPASS, replica_groups=[list(range(n_cores))],
                ins=[gb1.opt()], outs=[gt1.opt()])
            w_inT_full = pp.tile([128, F], BF16, tag="winF")
            nc.gpsimd.dma_start(
                w_inT_full[:], gt1.rearrange("(k z) f -> z k f", k=n_cores))

            # stream + matvec W3_wout
            mv_chunk(0, w3woutT_ap, b3wout_sb, w_outT_loc, 1, psA)
            mv_chunk(1, w3woutT_ap, b3wout_sb, w_outT_loc, 1, psB)

            # ---- pack + AllGather #2 (issued before the h pre-compute so
            # the collective isn't gated behind 25us of tanh work) --------
            sg_pack = wp.tile([128, 2], BF16, tag="sgp")
            for x in range(2):
                c0 = x * 128
                ptr = psA([128, 128], BF16)
                nc.tensor.transpose(ptr[:], w_outT_loc[:, c0:c0 + 128], eye_sb[:])
                wog = wp.tile([128, 128], BF16, tag="wog")
                nc.vector.tensor_scalar_mul(wog[:], ptr[:], gateF[:, x:x + 1])
                pti = psA([128, 128], BF16)
                nc.tensor.transpose(pti[:], w_inT_loc[:, c0:c0 + 128], eye_sb[:])
                wif = wp.tile([128, 128], BF16, tag="wif")
                nc.vector.tensor_copy(wif[:], pti[:])
                prod = wp.tile([128, 128], F32, tag="sprod")
                nc.vector.tensor_mul(prod[:], wif[:], wog[:])
                sgf = wp.tile([128, 1], F32, tag="sgf")
                nc.vector.tensor_reduce(sgf[:], prod[:], mybir.AxisListType.X, ADD)
                nc.vector.tensor_copy(sg_pack[:, x:x + 1], sgf[:])
                nc.gpsimd.dma_start(gb2[2 + c0:2 + c0 + 128, :], wog[:])
            nc.gpsimd.dma_start(gb2[0:2, :].rearrange("r f -> f r"), sg_pack[:])
            nc.gpsimd.collective_compute(
                "AllGather", BYPASS, replica_groups=[list(range(n_cores))],
                ins=[gb2.opt()], outs=[gt2.opt()])
            v2 = gt2.rearrange("(k r) z -> r k z", k=n_cores)
            w_outgT = [None, None]
            for x in range(2):
                w_outgT[x] = pp.tile([128, n_cores * 128], BF16,
                                     tag=f"wogg{x}", name=f"wogg{x}")
                nc.gpsimd.dma_start(w_outgT[x][:],
                                    v2[2 + x * 128:2 + (x + 1) * 128])
            v2s = gt2.rearrange("(k r) z -> z r k", k=n_cores)
            sg_full = pp.tile([128, 2 * n_cores], BF16, tag="sgfull")
            for x in range(2):  # col = x*8+k
                nc.gpsimd.dma_start(sg_full[:, x * n_cores:(x + 1) * n_cores],
                                    v2s[:, x, :])

            # h = tanh(z @ w_inT + b), all f-blocks, both batch halves at
            # once (1024-wide activations out of ping-ponged 2-bank psums).
            hstore = pp.tile([128, NFB * BL], BF16, tag="hstore")

            def part1(x):
                for k in range(n_cores):
                    blk = k * FL + x * 128
                    ph = psA([128, BL], F32) if k % 2 else psB([128, BL], F32)
                    for j in range(2):
                        nc.tensor.matmul(ph[:, j * BC:(j + 1) * BC],
                                         w_inT_full[:, blk:blk + 128],
                                         zt_sb[:, j * BC:(j + 1) * BC],
                                         start=True, stop=True)
                    idx = x * n_cores + k
                    nc.scalar.activation(hstore[:, idx * BL:(idx + 1) * BL],
                                         ph[:], TANH,
                                         bias=b_full[:, k * 2 + x:k * 2 + x + 1])

            part1(0)
            part1(1)

            # trace constant cneg = -sum_f sg (sg already gate/F-folded)
            sgs = wp.tile([128, 1], F32, tag="sgs")
            nc.vector.tensor_reduce(sgs[:], sg_full[:], mybir.AxisListType.X, ADD)
            sgs_bf = wp.tile([128, 1], BF16, tag="sgsbf")
            nc.vector.tensor_copy(sgs_bf[:], sgs[:])
            cps = psB([1, 1], F32)
            nc.tensor.matmul(cps[:], sgs_bf[:], ones_sb[:], start=True, stop=True)
            cneg = pp.tile([1, 1], F32, tag="cneg")
            nc.scalar.mul(cneg[:], cps[:], -1.0)

            # ---- phase 2 tail: dz / trace accumulation ------------------
            pdz = [ps_dz.tile([128, BC], F32, tag=f"pdz{j}", name=f"pdz{j}")
                   for j in range(2)]
            pt2 = [ps_t2.tile([1, BC], F32, tag=f"pt{j}", name=f"pt{j}")
                   for j in range(2)]
            for x in range(2):
                for k in range(n_cores):
                    idx = x * n_cores + k
                    first = (x == 0 and k == 0)
                    last = (x == 1 and k == n_cores - 1)
                    for j in range(2):
                        hsl = hstore[:, idx * BL + j * BC:idx * BL + (j + 1) * BC]
                        h2 = wp.tile([128, BC], BF16, tag="h2")
                        eng = nc.vector if (k + j) % 2 else nc.gpsimd
                        eng.tensor_mul(h2[:], hsl, hsl)
                        nc.tensor.matmul(pdz[j][:],
                                         w_outgT[x][:, k * 128:(k + 1) * 128],
                                         hsl, start=first, stop=last)
                        nc.tensor.matmul(pt2[j][:],
                                         sg_full[:, x * n_cores + k:
                                                 x * n_cores + k + 1],
                                         h2[:], start=first, stop=last)

            if dump:
                nc.sync.dma_start(d_winT[:], w_inT_full[:])
                nc.sync.dma_start(d_b[:], b_full[:])
                nc.sync.dma_start(d_sg[:], sg_full[:])
                nc.sync.dma_start(d_wog[:, 0:1024], w_outgT[0][:])
                nc.sync.dma_start(d_wog[:, 1024:2048], w_outgT[1][:])
                nc.sync.dma_start(d_h[:, 0:BL], hstore[:, 0:BL])
                nc.sync.dma_start(d_h[:, BL:2 * BL],
                                  hstore[:, n_cores * BL:(n_cores + 1) * BL])
                nc.sync.dma_start(d_winL[:], w_inT_loc[:])
                nc.sync.dma_start(d_woutL[:], w_outT_loc[:])
            for j in range(2):
                dz_sb = wp.tile([128, BC], F32, tag="dzsb")
                nc.vector.tensor_copy(dz_sb[:], pdz[j][:])
                nc.sync.dma_start(out_ap[0:Z, j * BC:(j + 1) * BC], dz_sb[:])
                tr_sb = wp.tile([1, BC], F32, tag="trsb")
                nc.vector.tensor_scalar_add(tr_sb[:], pt2[j][:], cneg[0:1, 0:1])
                nc.gpsimd.dma_start(out_ap[Z:Z + 1, j * BC:(j + 1) * BC],
                                    tr_sb[:])

    nc.compile()
    return nc


def host_prep(t, z_and_logpz, W1, B1, W2, B2, W3_win, b3_win,
              W3_wout, b3_wout, W3_b, b3_b, W3_gate, b3_gate,
              n_cores=N_CORES):
    """Shard + lay out the numpy inputs into per-core in_maps."""

    def col8(x):  # [4, 256] -> [128, 8] with col = k*2 + nb
        return np.ascontiguousarray(
            np.asarray(x, np.float32).reshape(4, 2, 128).transpose(2, 0, 1)
            .reshape(128, 8))

    t_in = np.asarray(t, np.float32).reshape(1, 1)
    w1c = col8(np.asarray(W1, np.float32)[:, :, 0])
    b1c = col8(B1)
    b2c = col8(B2)
    w2tc = np.ascontiguousarray(
        np.asarray(W2, np.float32).transpose(0, 2, 1)
        .reshape(4, 2, 128, 256).transpose(2, 0, 1, 3).reshape(128, 2048)).astype(BF)
    w3win_bf = np.asarray(W3_win, np.float32).astype(BF)
    w3wout_bf = np.asarray(W3_wout, np.float32).astype(BF)
    w3b_full = np.ascontiguousarray(np.asarray(W3_b, np.float32).astype(BF).T)
    b3b_full = np.ascontiguousarray(
        np.asarray(b3_b, np.float32).reshape(NFB, 128).T)
    w3gate_bf = np.asarray(W3_gate, np.float32).astype(BF)
    b3win = np.asarray(b3_win, np.float32)
    b3wout = np.asarray(b3_wout, np.float32)
    b3gate = np.asarray(b3_gate, np.float32)
    z = np.asarray(z_and_logpz, np.float32)[:, :Z]
    ztb = np.ascontiguousarray(z.T).astype(BF)
    eye = np.eye(128, dtype=np.float32).astype(BF)
    ones = np.ones((128, 1), dtype=np.float32).astype(BF)

    in_maps = []
    for k in range(n_cores):
        r0 = k * RPC
        f0 = k * FL
        in_maps.append({
            "t": t_in, "w1c": w1c, "b1c": b1c, "b2c": b2c, "w2tc": w2tc,
            "w3winT_sl": np.ascontiguousarray(w3win_bf[r0:r0 + RPC].T),
            "w3woutT_sl": np.ascontiguousarray(w3wout_bf[r0:r0 + RPC].T),
            "b3win_c": np.ascontiguousarray(
                b3win[r0:r0 + RPC].reshape(FL, 128).T),
            "b3wout_c": np.ascontiguousarray(
                b3wout[r0:r0 + RPC].reshape(FL, 128).T),
            "w3bT_full": w3b_full, "b3b_full": b3b_full,
            "w3gateT_sl": np.ascontiguousarray(w3gate_bf[f0:f0 + FL].T),
            "b3gate_c": np.ascontiguousarray(
                b3gate[f0:f0 + FL].reshape(2, 128).T),
            "ztb": np.ascontiguousarray(ztb[:, k * BL:(k + 1) * BL]),
            "eyeb": eye, "onesb": ones,
        })
    return in_maps


_NC_CACHE = {}


def kernel(**inputs) -> np.ndarray:
    _ensure_ntff_hook()
    from concourse import bass_utils

    key = "full"
    if key not in _NC_CACHE:
        _NC_CACHE[key] = build_module()
    nc = _NC_CACHE[key]

    in_maps = host_prep(**inputs)
    res = bass_utils.run_bass_kernel_spmd(nc, in_maps, list(range(N_CORES)))
    out = np.empty((B, Z + 1), np.float32)
    for k in range(N_CORES):
        out[k * BL:(k + 1) * BL, :] = res.results[k]["out"].T
    return out
